# revision 24
# baseline (speedup 1.0000x reference)
"""Complex multihead attention (split softmax) on 8 Trainium2 NeuronCores.

Sharding: data-parallel over batch (B=4) x tensor-parallel over heads
(16 heads -> 2 groups of 8). core = b*2 + head_group.

v2 design (bf16 wire + pipelined attention):
 - All matmul operands bf16 (same PE rate as fp32r at N>=512, but halves
   HBM traffic, SBUF footprint, and enables FWL weight loads).
 - K bias dropped (softmax shift invariant), V bias folded host-side,
   Q bias applied at projection evacuation.
 - Q/K evacuation builds the score-matmul operand variants directly:
   qplain=[Qr+br;Qi+bi], qswap=[Qi+bi;Qr+br] (partition-swap DMA),
   kneg=[Kr;-Ki] (ACT with +/-1 per-partition scale), kplain=[Kr;Ki].
   st_r = kneg^T qplain, st_i = kplain^T qswap. No per-head prep in the
   attention loop -> no PE bubbles between heads.
 - Both score tiles share ONE bf16 PSUM bank [128,1024]; exp runs as a
   single [128,1024] ACT op (saves ~300ns/step of ACT overhead).
 - Softmax denominators via ones[128,128] matmuls -> sums replicated on
   all 128 partitions (no reciprocal-broadcast DMA in finalize).
 - PSUM budget: st 2 banks (double-buffered) + sums 3 + PV accum 3 = 8.
 - wo prefetched during attention as host-pretransposed contiguous tiles.
"""

import numpy as np
import ml_dtypes

import concourse.bass as bass
from concourse import bacc
import concourse.mybir as mybir
import concourse.tile as tile
from concourse.bass_utils import run_bass_kernel_spmd

S, B, E, H, D = 1024, 4, 1024, 16, 64
HPC = 8            # heads per core
EH = HPC * D       # 512
N_CORES = 8
F32 = mybir.dt.float32
BF = mybir.dt.bfloat16
AF = mybir.ActivationFunctionType
BF_NP = ml_dtypes.bfloat16

_NC_CACHE = []


def _emit(tc):
    nc = tc.nc
    xq = nc.dram_tensor("xq", [2 * E, S], BF, kind="ExternalInput").ap()
    xk = nc.dram_tensor("xk", [2 * E, S], BF, kind="ExternalInput").ap()
    xv = nc.dram_tensor("xv", [2 * E, S], BF, kind="ExternalInput").ap()
    wq = nc.dram_tensor("wq", [2 * E, 2 * EH], BF, kind="ExternalInput").ap()
    wk = nc.dram_tensor("wk", [2 * E, 2 * EH], BF, kind="ExternalInput").ap()
    wv = nc.dram_tensor("wv", [2 * E, 2 * EH], BF, kind="ExternalInput").ap()
    wod0 = nc.dram_tensor("wod0", [8, 128, 1024], BF, kind="ExternalInput").ap()
    wod1 = nc.dram_tensor("wod1", [8, 128, 1024], BF, kind="ExternalInput").ap()
    bq = nc.dram_tensor("bq", [128, HPC], F32, kind="ExternalInput").ap()
    spm = nc.dram_tensor("spm", [128, 1], F32, kind="ExternalInput").ap()
    onesd = nc.dram_tensor("onesd", [128, 128], BF, kind="ExternalInput").ap()
    ytr = nc.dram_tensor("ytr", [E, S], F32, kind="ExternalOutput").ap()
    yti = nc.dram_tensor("yti", [E, S], F32, kind="ExternalOutput").ap()

    store = tc.alloc_tile_pool(name="store", bufs=1)
    qplain = store.tile([128, HPC * S], BF)
    qswap = store.tile([128, HPC * S], BF)
    kneg = store.tile([128, HPC * S], BF)
    kplain = store.tile([128, HPC * S], BF)
    vnat = store.tile([128, 8 * 1024], BF)
    vsw = store.tile([128, 8 * 1024], BF)
    attn = store.tile([128, HPC * S], BF)
    bq_sb = store.tile([128, HPC], F32)
    spm_sb = store.tile([128, 1], F32)
    ones_sb = store.tile([128, 128], BF)

    # ---------------- projections ----------------
    with tc.tile_pool(name="xp", bufs=16) as xp, \
         tc.tile_pool(name="wp", bufs=4) as wp, \
         tc.tile_pool(name="pp", bufs=8, space="PSUM") as pp:

        for which, xdram, wdram in (("q", xq, wq), ("k", xk, wk)):
            xs = []
            w0 = []
            for k in range(16):
                wt = wp.tile([128, 512], BF, tag="w", name=f"w{which}0{k}")
                nc.sync.dma_start(
                    out=wt, in_=wdram[k * 128:(k + 1) * 128, 0:512])
                w0.append(wt)
                xt = xp.tile([128, S], BF, tag="x", name=f"x{which}{k}")
                if which == "q":
                    # split so the hf=0 matmuls unblock after half a tile
                    nc.sync.dma_start(
                        out=xt[:, 0:512],
                        in_=xdram[k * 128:(k + 1) * 128, 0:512])
                    nc.sync.dma_start(
                        out=xt[:, 512:1024],
                        in_=xdram[k * 128:(k + 1) * 128, 512:1024])
                else:
                    nc.sync.dma_start(
                        out=xt, in_=xdram[k * 128:(k + 1) * 128, :])
                xs.append(xt)
                if which == "q" and k == 0:
                    # small constants: needed only by evacs much later
                    nc.sync.dma_start(out=bq_sb, in_=bq)
                    nc.sync.dma_start(out=spm_sb, in_=spm)
                    nc.sync.dma_start(out=ones_sb, in_=onesd)
            for grp in range(2):
                if grp == 0:
                    wts = w0
                else:
                    wts = []
                    for k in range(16):
                        wt = wp.tile([128, 512], BF, tag="w",
                                     name=f"w{which}1{k}")
                        nc.sync.dma_start(
                            out=wt,
                            in_=wdram[k * 128:(k + 1) * 128, 512:1024])
                        wts.append(wt)
                ps = [[pp.tile([128, 512], F32, tag="pp",
                               name=f"p{which}{grp}{j}{hf}")
                       for hf in range(2)] for j in range(4)]
                for k in range(16):
                    for j in range(4):
                        lhsT = wts[k][:, j * 128:(j + 1) * 128]
                        for hf in range(2):
                            nc.tensor.matmul(
                                ps[j][hf], lhsT,
                                xs[k][:, hf * 512:(hf + 1) * 512],
                                start=(k == 0), stop=(k == 15))
                for j in range(4):
                    h = grp * 4 + j
                    for hf in range(2):
                        c0 = h * S + hf * 512
                        if which == "q":
                            # alternate engines to halve the serial evac chain
                            if hf == 0:
                                nc.scalar.activation(
                                    qplain[:, c0:c0 + 512], ps[j][hf],
                                    AF.Identity, bias=bq_sb[:, h:h + 1])
                            else:
                                nc.vector.tensor_scalar_add(
                                    qplain[:, c0:c0 + 512], ps[j][hf],
                                    bq_sb[:, h:h + 1])
                        else:
                            nc.scalar.activation(
                                kneg[:, c0:c0 + 512], ps[j][hf],
                                AF.Identity, scale=spm_sb[:, 0:1])
                            nc.vector.tensor_copy(
                                kplain[:, c0:c0 + 512], ps[j][hf])
                    if which == "q":
                        nc.sync.dma_start(
                            out=qswap[0:64, h * S:(h + 1) * S],
                            in_=qplain[64:128, h * S:(h + 1) * S])
                        nc.sync.dma_start(
                            out=qswap[64:128, h * S:(h + 1) * S],
                            in_=qplain[0:64, h * S:(h + 1) * S])

        # V projection, natural layout: vnat[s, t*1024 + (j, r/i, d)]
        xs = []
        wv0 = []
        for k in range(16):
            wt = wp.tile([128, 512], BF, tag="w", name=f"wv0{k}")
            nc.sync.dma_start(out=wt, in_=wv[k * 128:(k + 1) * 128, 0:512])
            wv0.append(wt)
            xt = xp.tile([128, S], BF, tag="x", name=f"xv{k}")
            nc.sync.dma_start(out=xt, in_=xv[k * 128:(k + 1) * 128, :])
            xs.append(xt)
        for hf in range(2):
            if hf == 0:
                wts = wv0
            else:
                wts = []
                for k in range(16):
                    wt = wp.tile([128, 512], BF, tag="w", name=f"wv1{k}")
                    nc.sync.dma_start(
                        out=wt, in_=wv[k * 128:(k + 1) * 128, 512:1024])
                    wts.append(wt)
            ps = [pp.tile([128, 512], F32, tag="pp", name=f"pv{hf}{st}")
                  for st in range(8)]
            for k in range(16):
                for st in range(8):
                    nc.tensor.matmul(ps[st],
                                     xs[k][:, st * 128:(st + 1) * 128],
                                     wts[k][:, :],
                                     start=(k == 0), stop=(k == 15))
            # evac: ONE op per PSUM bank (frees banks for the attention
            # pools fast), alternating engines; vsw built from SBUF off-path
            for st in range(8):
                base = st * 1024 + hf * 512
                if st % 2 == 0:
                    nc.vector.tensor_copy(vnat[:, base:base + 512], ps[st])
                else:
                    nc.scalar.copy(vnat[:, base:base + 512], ps[st])
            for st in range(8):
                base = st * 1024 + hf * 512
                sv = vnat[:, base:base + 512].rearrange(
                    "p (j r d) -> p j r d", j=4, r=2, d=64)
                dv = vsw[:, base:base + 512].rearrange(
                    "p (j r d) -> p j r d", j=4, r=2, d=64)
                nc.vector.tensor_copy(dv[:, :, 0, :], sv[:, :, 1, :])
                nc.vector.tensor_copy(dv[:, :, 1, :], sv[:, :, 0, :])

    # O-projection weights prefetch (contiguous tiles, DMA idles in attn)
    wop = tc.alloc_tile_pool(name="wop", bufs=1)
    wo_tiles = {}
    for part, wo_d in ((0, wod0), (1, wod1)):
        for m in range(8):
            wt = wop.tile([128, 1024], BF, tag=f"wo{part}{m}",
                          name=f"wo{part}{m}")
            nc.sync.dma_start(out=wt, in_=wo_d[m])
            wo_tiles[(part, m)] = wt

    # ---------------- attention ----------------
    with tc.tile_pool(name="asb", bufs=2) as asb, \
         tc.tile_pool(name="ptp", bufs=3) as ptp, \
         tc.tile_pool(name="stp", bufs=2, space="PSUM") as stp, \
         tc.tile_pool(name="smp", bufs=2, space="PSUM") as smp, \
         tc.tile_pool(name="opp", bufs=2, space="PSUM") as opp:

        def finalize(fin):
            """Emit at the start of the NEXT (j,qi) iter. DVE op order
            mirrors the next iter's PSUM slot reuse order so slots free
            just-in-time: rcp halves (free s_pair), then o-copies (o1,o2).
            Sum matmuls put sr on partitions 0-63 and si on 64-127; the
            reciprocal halves are broadcast to the other half via DMA."""
            j, qi, s_pair, o1, o2 = fin
            c0 = j * S + qi * 512
            # full-tile reciprocal (custom-DVE op needs base partition 0):
            # rcp = [1/sr (top); 1/si (bottom)]; bc = crossed halves via DMA
            rcp = asb.tile([128, 512], F32, tag="rcp", name=f"rp{j}{qi}")
            nc.vector.reciprocal_approx_fast(rcp, s_pair)
            bc = asb.tile([128, 512], F32, tag="bc", name=f"bc{j}{qi}")
            nc.sync.dma_start(out=bc[64:128, :], in_=rcp[0:64, :])
            nc.sync.dma_start(out=bc[0:64, :], in_=rcp[64:128, :])
            oc1 = asb.tile([128, 512], F32, tag="oc", name=f"oc1{j}{qi}")
            oc2 = asb.tile([128, 512], F32, tag="oc", name=f"oc2{j}{qi}")
            nc.vector.tensor_copy(oc1, o1)
            nc.vector.tensor_copy(oc2, o2)
            t1 = asb.tile([128, 512], F32, tag="tt", name=f"t1{j}{qi}")
            t2 = asb.tile([128, 512], F32, tag="tt", name=f"t2{j}{qi}")
            nc.vector.tensor_mul(t1[0:64, :], oc1[0:64, :], rcp[0:64, :])
            nc.vector.tensor_mul(t1[64:128, :], oc1[64:128, :],
                                 bc[64:128, :])
            nc.vector.tensor_mul(t2[0:64, :], oc2[0:64, :], bc[0:64, :])
            nc.vector.tensor_mul(t2[64:128, :], oc2[64:128, :],
                                 rcp[64:128, :])
            dst = attn[:, c0:c0 + 512]
            nc.vector.tensor_sub(dst[0:64, :], t1[0:64, :], t2[0:64, :])
            nc.vector.tensor_add(dst[64:128, :], t1[64:128, :], t2[64:128, :])

        steps = [(j, qi, t)
                 for j in range(HPC) for qi in range(2) for t in range(8)]

        def emit_st(n):
            j, qi, t = steps[n]
            st = stp.tile([128, 1024], F32, tag="st", name=f"st{n}")
            q0 = j * S + qi * 512
            k0 = j * S + t * 128
            nc.tensor.matmul(st[:, 0:512], kneg[:, k0:k0 + 128],
                             qplain[:, q0:q0 + 512], start=True, stop=True)
            nc.tensor.matmul(st[:, 512:1024], kplain[:, k0:k0 + 128],
                             qswap[:, q0:q0 + 512], start=True, stop=True)
            return st

        def consumers(j, qi, t, pt, acc):
            s_pair, o1, o2 = acc
            st_, sp_ = (t == 0), (t == 7)
            # col-tiled pair shares one bank and runs concurrently
            # (per-region has_written semantics verified on HW)
            nc.tensor.matmul(s_pair[0:64, :], ones_sb[:, 0:64],
                             pt[:, 0:512], start=st_, stop=sp_,
                             skip_group_check=True)
            nc.tensor.matmul(s_pair[64:128, :], ones_sb[:, 64:128],
                             pt[:, 512:1024], start=st_, stop=sp_,
                             skip_group_check=True)
            vbase = t * 1024 + j * 128
            nc.tensor.matmul(o1, vnat[:, vbase:vbase + 128], pt[:, 0:512],
                             start=st_, stop=sp_)
            nc.tensor.matmul(o2, vsw[:, vbase:vbase + 128],
                             pt[:, 512:1024], start=st_, stop=sp_)

        pending = None
        cur = None
        held = None  # t=0 consumers deferred to t=1 (slot-reuse slack)
        sts = {0: emit_st(0), 1: emit_st(1)}
        for n, (j, qi, t) in enumerate(steps):
            pt = ptp.tile([128, 1024], BF, tag="pt", name=f"pt{n}")
            nc.scalar.activation(pt, sts.pop(n), AF.Exp, scale=0.125)
            if n + 2 < len(steps):
                sts[n + 2] = emit_st(n + 2)
            if t == 0:
                if pending is not None:
                    finalize(pending)
                    pending = None
                s_pair = smp.tile([128, 512], F32, tag="sa", name=f"sp{j}{qi}")
                o1 = opp.tile([128, 512], F32, tag="ov", name=f"o1_{j}{qi}")
                o2 = opp.tile([128, 512], F32, tag="ov", name=f"o2_{j}{qi}")
                cur = (s_pair, o1, o2)
                held = (j, qi, t, pt)
                continue
            if held is not None:
                hj, hqi, ht, hpt = held
                consumers(hj, hqi, ht, hpt, cur)
                held = None
            consumers(j, qi, t, pt, cur)
            if t == 7:
                pending = (steps[n][0], steps[n][1], *cur)
        finalize(pending)

    # ---------------- O projection (partials) ----------------
    with tc.tile_pool(name="ytp", bufs=4) as ytp, \
         tc.tile_pool(name="pop", bufs=4, space="PSUM") as pop:
        for part, yt_d in ((0, ytr), (1, yti)):
            for m in range(8):
                wt = wo_tiles[(part, m)]
                for hf in range(2):
                    po = pop.tile([128, 512], F32, tag="po",
                                  name=f"po{part}{m}{hf}")
                    for jj in range(8):
                        nc.tensor.matmul(
                            po, wt[:, jj * 128:(jj + 1) * 128],
                            attn[:, jj * S + hf * 512: jj * S + (hf + 1) * 512],
                            start=(jj == 0), stop=(jj == 7))
                    yt_t = ytp.tile([128, 512], F32, tag="yt",
                                    name=f"yt{part}{m}{hf}")
                    nc.vector.tensor_copy(yt_t, po)
                    nc.sync.dma_start(
                        out=yt_d[m * 128:(m + 1) * 128,
                                 hf * 512:(hf + 1) * 512],
                        in_=yt_t)

    wop.release()
    store.release()


def build_module():
    nc = bacc.Bacc("TRN2", target_bir_lowering=False)
    with tile.TileContext(nc) as tc:
        _emit(tc)
    nc.compile()
    return nc


def _get_nc():
    if not _NC_CACHE:
        _NC_CACHE.append(build_module())
    return _NC_CACHE[0]


def prep_core(inp, core):
    """Host-side shard prep for one core (bf16 wire format)."""
    b, hg = divmod(core, 2)
    hs, he = hg * EH, (hg + 1) * EH

    def xcat(xr, xi):
        return np.ascontiguousarray(
            np.concatenate([xr[:, b, :].T, xi[:, b, :].T], axis=0)
        ).astype(BF_NP)

    def w_prep(wr, wi):
        A = wr[hs:he, :].T
        Bm = wi[hs:he, :].T
        top = np.concatenate(
            [A.reshape(E, HPC, D), Bm.reshape(E, HPC, D)], axis=2)
        bot = np.concatenate(
            [-Bm.reshape(E, HPC, D), A.reshape(E, HPC, D)], axis=2)
        return np.ascontiguousarray(
            np.concatenate([top.reshape(E, 2 * EH), bot.reshape(E, 2 * EH)],
                           axis=0)).astype(BF_NP)

    def wo_prep(w_top, w_bot):
        Ct = w_top[:, hs:he].T.reshape(HPC, D, E)
        Dt = w_bot[:, hs:he].T.reshape(HPC, D, E)
        packed = np.concatenate([Ct, Dt], axis=1).reshape(2 * EH, E)
        A2 = packed.reshape(HPC, 128, E)  # [j, p, e]
        out = np.empty((8, 128, 1024), np.float32)
        for m in range(8):
            out[m] = A2[:, :, m * 128:(m + 1) * 128].transpose(
                1, 0, 2).reshape(128, 1024)
        return out.astype(BF_NP)

    bqp = np.empty((128, HPC), np.float32)
    for j in range(HPC):
        h = hg * HPC + j
        bqp[:64, j] = inp["bq_r"][h * D:(h + 1) * D]
        bqp[64:, j] = inp["bq_i"][h * D:(h + 1) * D]

    spm = np.concatenate([np.ones((64, 1), np.float32),
                          -np.ones((64, 1), np.float32)], axis=0)

    return dict(
        xq=xcat(inp["query_r"], inp["query_i"]),
        xk=xcat(inp["key_r"], inp["key_i"]),
        xv=xcat(inp["value_r"], inp["value_i"]),
        wq=w_prep(inp["wq_r"], inp["wq_i"]),
        wk=w_prep(inp["wk_r"], inp["wk_i"]),
        wv=w_prep(inp["wv_r"], inp["wv_i"]),
        wod0=wo_prep(inp["wo_r"], -inp["wo_i"]),
        wod1=wo_prep(inp["wo_i"], inp["wo_r"]),
        bq=bqp,
        spm=spm,
        onesd=np.ones((128, 128), np.float32).astype(BF_NP),
    )


def host_combine(results, inp):
    """Sum per-core partials, add the host-side constant, untranspose."""
    bvr = inp["bv_r"].astype(np.float64)
    bvi = inp["bv_i"].astype(np.float64)
    wr = inp["wo_r"].astype(np.float64)
    wi = inp["wo_i"].astype(np.float64)
    vb_r = bvr - bvi
    vb_i = bvr + bvi
    yc_r = (wr @ vb_r - wi @ vb_i + inp["bo_r"]).astype(np.float32)
    yc_i = (wr @ vb_i + wi @ vb_r + inp["bo_i"]).astype(np.float32)

    out = np.empty((S, B, E, 2), np.float32)
    for b in range(B):
        yr = results[2 * b]["ytr"] + results[2 * b + 1]["ytr"]
        yi = results[2 * b]["yti"] + results[2 * b + 1]["yti"]
        out[:, b, :, 0] = yr.T + yc_r
        out[:, b, :, 1] = yi.T + yc_i
    return out


def kernel(**inputs):
    inputs = {k: np.asarray(v) for k, v in inputs.items()}
    nc = _get_nc()
    in_maps = [prep_core(inputs, c) for c in range(N_CORES)]
    res = run_bass_kernel_spmd(nc, in_maps, core_ids=list(range(N_CORES)))
    return host_combine(res.results, inputs)


# revision 29
# speedup vs baseline: 1.0345x; 1.0345x over previous
"""Complex multihead attention (split softmax) on 8 Trainium2 NeuronCores.

Sharding: data-parallel over batch (B=4) x tensor-parallel over heads
(16 heads -> 2 groups of 8). core = b*2 + head_group.

v2 design (bf16 wire + pipelined attention):
 - All matmul operands bf16 (same PE rate as fp32r at N>=512, but halves
   HBM traffic, SBUF footprint, and enables FWL weight loads).
 - K bias dropped (softmax shift invariant), V bias folded host-side,
   Q bias applied at projection evacuation.
 - Q/K evacuation builds the score-matmul operand variants directly:
   qplain=[Qr+br;Qi+bi], qswap=[Qi+bi;Qr+br] (partition-swap DMA),
   kneg=[Kr;-Ki] (ACT with +/-1 per-partition scale), kplain=[Kr;Ki].
   st_r = kneg^T qplain, st_i = kplain^T qswap. No per-head prep in the
   attention loop -> no PE bubbles between heads.
 - Both score tiles share ONE bf16 PSUM bank [128,1024]; exp runs as a
   single [128,1024] ACT op (saves ~300ns/step of ACT overhead).
 - Softmax denominators via ones[128,128] matmuls -> sums replicated on
   all 128 partitions (no reciprocal-broadcast DMA in finalize).
 - PSUM budget: st 2 banks (double-buffered) + sums 3 + PV accum 3 = 8.
 - wo prefetched during attention as host-pretransposed contiguous tiles.
"""

import numpy as np
import ml_dtypes

import concourse.bass as bass
from concourse import bacc
import concourse.mybir as mybir
import concourse.tile as tile
from concourse.bass_utils import run_bass_kernel_spmd

S, B, E, H, D = 1024, 4, 1024, 16, 64
HPC = 8            # heads per core
EH = HPC * D       # 512
N_CORES = 8
F32 = mybir.dt.float32
BF = mybir.dt.bfloat16
AF = mybir.ActivationFunctionType
BF_NP = ml_dtypes.bfloat16

_NC_CACHE = []


def _emit(tc):
    nc = tc.nc
    xq = nc.dram_tensor("xq", [2 * E, S], BF, kind="ExternalInput").ap()
    xk = nc.dram_tensor("xk", [2 * E, S], BF, kind="ExternalInput").ap()
    xv = nc.dram_tensor("xv", [2 * E, S], BF, kind="ExternalInput").ap()
    wq = nc.dram_tensor("wq", [2 * E, 2 * EH], BF, kind="ExternalInput").ap()
    wk = nc.dram_tensor("wk", [2 * E, 2 * EH], BF, kind="ExternalInput").ap()
    wv = nc.dram_tensor("wv", [2 * E, 2 * EH], BF, kind="ExternalInput").ap()
    wod0 = nc.dram_tensor("wod0", [8, 128, 1024], BF, kind="ExternalInput").ap()
    wod1 = nc.dram_tensor("wod1", [8, 128, 1024], BF, kind="ExternalInput").ap()
    bq = nc.dram_tensor("bq", [128, HPC], F32, kind="ExternalInput").ap()
    spm = nc.dram_tensor("spm", [128, 1], F32, kind="ExternalInput").ap()
    onesd = nc.dram_tensor("onesd", [128, 128], BF, kind="ExternalInput").ap()
    ytr = nc.dram_tensor("ytr", [E, S], F32, kind="ExternalOutput").ap()
    yti = nc.dram_tensor("yti", [E, S], F32, kind="ExternalOutput").ap()

    store = tc.alloc_tile_pool(name="store", bufs=1)
    qplain = store.tile([128, HPC * S], BF)
    qswap = store.tile([128, HPC * S], BF)
    kneg = store.tile([128, HPC * S], BF)
    kplain = store.tile([128, HPC * S], BF)
    vnat = store.tile([128, 8 * 1024], BF)
    vsw = store.tile([128, 8 * 1024], BF)
    attn = store.tile([128, HPC * S], BF)
    bq_sb = store.tile([128, HPC], F32)
    spm_sb = store.tile([128, 1], F32)
    ones_sb = store.tile([128, 128], BF)
    # PE warmup: HAM un-throttles after ~3.4us of sustained matmul activity;
    # burn the input-DMA wait on dummy matmuls so real work starts warm.
    # (scratch rhs is uninitialized; the PSUM result is never read)
    nc.sync.dma_start(out=ones_sb, in_=onesd)
    wu_rhs = store.tile([128, 512], BF)
    nc.vector.memset(wu_rhs, 0.0)
    with tc.tile_pool(name="wup", bufs=1, space="PSUM") as wup:
        wu = wup.tile([128, 512], F32)
        for i in range(20):
            nc.tensor.matmul(wu, ones_sb, wu_rhs,
                             start=(i == 0), stop=(i == 19))

    # ---------------- projections ----------------
    with tc.tile_pool(name="xp", bufs=16) as xp, \
         tc.tile_pool(name="wp", bufs=4) as wp, \
         tc.tile_pool(name="pp", bufs=8, space="PSUM") as pp:

        for which, xdram, wdram in (("q", xq, wq), ("k", xk, wk)):
            xs = []
            w0 = []
            for k in range(16):
                wt = wp.tile([128, 512], BF, tag="w", name=f"w{which}0{k}")
                nc.scalar.dma_start(
                    out=wt, in_=wdram[k * 128:(k + 1) * 128, 0:512])
                w0.append(wt)
                xt = xp.tile([128, S], BF, tag="x", name=f"x{which}{k}")
                if which == "q":
                    # split so the hf=0 matmuls unblock after half a tile
                    nc.sync.dma_start(
                        out=xt[:, 0:512],
                        in_=xdram[k * 128:(k + 1) * 128, 0:512])
                    nc.sync.dma_start(
                        out=xt[:, 512:1024],
                        in_=xdram[k * 128:(k + 1) * 128, 512:1024])
                else:
                    nc.sync.dma_start(
                        out=xt, in_=xdram[k * 128:(k + 1) * 128, :])
                xs.append(xt)
                if which == "q" and k == 0:
                    # small constants: needed only by evacs much later
                    nc.sync.dma_start(out=bq_sb, in_=bq)
                    nc.sync.dma_start(out=spm_sb, in_=spm)
            for grp in range(2):
                if grp == 0:
                    wts = w0
                else:
                    wts = []
                    for k in range(16):
                        wt = wp.tile([128, 512], BF, tag="w",
                                     name=f"w{which}1{k}")
                        nc.scalar.dma_start(
                            out=wt,
                            in_=wdram[k * 128:(k + 1) * 128, 512:1024])
                        wts.append(wt)
                ps = [[pp.tile([128, 512], F32, tag="pp",
                               name=f"p{which}{grp}{j}{hf}")
                       for hf in range(2)] for j in range(4)]
                for k in range(16):
                    for j in range(4):
                        lhsT = wts[k][:, j * 128:(j + 1) * 128]
                        for hf in range(2):
                            nc.tensor.matmul(
                                ps[j][hf], lhsT,
                                xs[k][:, hf * 512:(hf + 1) * 512],
                                start=(k == 0), stop=(k == 15))
                for j in range(4):
                    h = grp * 4 + j
                    for hf in range(2):
                        c0 = h * S + hf * 512
                        if which == "q":
                            # alternate engines to halve the serial evac chain
                            if hf == 0:
                                nc.scalar.activation(
                                    qplain[:, c0:c0 + 512], ps[j][hf],
                                    AF.Identity, bias=bq_sb[:, h:h + 1])
                            else:
                                nc.vector.tensor_scalar_add(
                                    qplain[:, c0:c0 + 512], ps[j][hf],
                                    bq_sb[:, h:h + 1])
                        else:
                            nc.scalar.activation(
                                kneg[:, c0:c0 + 512], ps[j][hf],
                                AF.Identity, scale=spm_sb[:, 0:1])
                            nc.vector.tensor_copy(
                                kplain[:, c0:c0 + 512], ps[j][hf])
                    if which == "q":
                        nc.sync.dma_start(
                            out=qswap[0:64, h * S:(h + 1) * S],
                            in_=qplain[64:128, h * S:(h + 1) * S])
                        nc.sync.dma_start(
                            out=qswap[64:128, h * S:(h + 1) * S],
                            in_=qplain[0:64, h * S:(h + 1) * S])

        # V projection, natural layout: vnat[s, t*1024 + (j, r/i, d)]
        xs = []
        wv0 = []
        for k in range(16):
            wt = wp.tile([128, 512], BF, tag="w", name=f"wv0{k}")
            nc.scalar.dma_start(out=wt, in_=wv[k * 128:(k + 1) * 128, 0:512])
            wv0.append(wt)
            xt = xp.tile([128, S], BF, tag="x", name=f"xv{k}")
            nc.sync.dma_start(out=xt, in_=xv[k * 128:(k + 1) * 128, :])
            xs.append(xt)
        for hf in range(2):
            if hf == 0:
                wts = wv0
            else:
                wts = []
                for k in range(16):
                    wt = wp.tile([128, 512], BF, tag="w", name=f"wv1{k}")
                    nc.scalar.dma_start(
                        out=wt, in_=wv[k * 128:(k + 1) * 128, 512:1024])
                    wts.append(wt)
            ps = [pp.tile([128, 512], F32, tag="pp", name=f"pv{hf}{st}")
                  for st in range(8)]
            for k in range(16):
                for st in range(8):
                    nc.tensor.matmul(ps[st],
                                     xs[k][:, st * 128:(st + 1) * 128],
                                     wts[k][:, :],
                                     start=(k == 0), stop=(k == 15))
            # evac: ONE op per PSUM bank (frees banks for the attention
            # pools fast), alternating engines; vsw built from SBUF off-path
            for st in range(8):
                base = st * 1024 + hf * 512
                if st % 2 == 0:
                    nc.vector.tensor_copy(vnat[:, base:base + 512], ps[st])
                else:
                    nc.scalar.copy(vnat[:, base:base + 512], ps[st])
            for st in range(8):
                base = st * 1024 + hf * 512
                sv = vnat[:, base:base + 512].rearrange(
                    "p (j r d) -> p j r d", j=4, r=2, d=64)
                dv = vsw[:, base:base + 512].rearrange(
                    "p (j r d) -> p j r d", j=4, r=2, d=64)
                nc.vector.tensor_copy(dv[:, :, 0, :], sv[:, :, 1, :])
                nc.vector.tensor_copy(dv[:, :, 1, :], sv[:, :, 0, :])

    # O-projection weights prefetch (contiguous tiles, DMA idles in attn)
    wop = tc.alloc_tile_pool(name="wop", bufs=1)
    wo_tiles = {}
    for part, wo_d in ((0, wod0), (1, wod1)):
        for m in range(8):
            wt = wop.tile([128, 1024], BF, tag=f"wo{part}{m}",
                          name=f"wo{part}{m}")
            nc.sync.dma_start(out=wt, in_=wo_d[m])
            wo_tiles[(part, m)] = wt

    # ---------------- attention ----------------
    with tc.tile_pool(name="asb", bufs=2) as asb, \
         tc.tile_pool(name="ptp", bufs=3) as ptp, \
         tc.tile_pool(name="stp", bufs=2, space="PSUM") as stp, \
         tc.tile_pool(name="smp", bufs=2, space="PSUM") as smp, \
         tc.tile_pool(name="opp", bufs=2, space="PSUM") as opp:

        def finalize(fin):
            """Emit at the start of the NEXT (j,qi) iter. DVE op order
            mirrors the next iter's PSUM slot reuse order so slots free
            just-in-time: rcp halves (free s_pair), then o-copies (o1,o2).
            Sum matmuls put sr on partitions 0-63 and si on 64-127; the
            reciprocal halves are broadcast to the other half via DMA."""
            j, qi, s_pair, o1, o2 = fin
            c0 = j * S + qi * 512
            # full-tile reciprocal (custom-DVE op needs base partition 0):
            # rcp = [1/sr (top); 1/si (bottom)]; bc = crossed halves via DMA
            rcp = asb.tile([128, 512], F32, tag="rcp", name=f"rp{j}{qi}")
            nc.vector.reciprocal_approx_fast(rcp, s_pair)
            bc = asb.tile([128, 512], F32, tag="bc", name=f"bc{j}{qi}")
            nc.sync.dma_start(out=bc[64:128, :], in_=rcp[0:64, :])
            nc.sync.dma_start(out=bc[0:64, :], in_=rcp[64:128, :])
            oc1 = asb.tile([128, 512], F32, tag="oc", name=f"oc1{j}{qi}")
            oc2 = asb.tile([128, 512], F32, tag="oc", name=f"oc2{j}{qi}")
            nc.vector.tensor_copy(oc1, o1)
            nc.vector.tensor_copy(oc2, o2)
            t1 = asb.tile([128, 512], F32, tag="tt", name=f"t1{j}{qi}")
            t2 = asb.tile([128, 512], F32, tag="tt", name=f"t2{j}{qi}")
            nc.vector.tensor_mul(t1[0:64, :], oc1[0:64, :], rcp[0:64, :])
            nc.vector.tensor_mul(t1[64:128, :], oc1[64:128, :],
                                 bc[64:128, :])
            nc.vector.tensor_mul(t2[0:64, :], oc2[0:64, :], bc[0:64, :])
            nc.vector.tensor_mul(t2[64:128, :], oc2[64:128, :],
                                 rcp[64:128, :])
            dst = attn[:, c0:c0 + 512]
            nc.vector.tensor_sub(dst[0:64, :], t1[0:64, :], t2[0:64, :])
            nc.vector.tensor_add(dst[64:128, :], t1[64:128, :], t2[64:128, :])

        steps = [(j, qi, t)
                 for j in range(HPC) for qi in range(2) for t in range(8)]

        def emit_st(n):
            j, qi, t = steps[n]
            st = stp.tile([128, 1024], F32, tag="st", name=f"st{n}")
            q0 = j * S + qi * 512
            k0 = j * S + t * 128
            nc.tensor.matmul(st[:, 0:512], kneg[:, k0:k0 + 128],
                             qplain[:, q0:q0 + 512], start=True, stop=True)
            nc.tensor.matmul(st[:, 512:1024], kplain[:, k0:k0 + 128],
                             qswap[:, q0:q0 + 512], start=True, stop=True)
            return st

        def consumers(j, qi, t, pt, acc):
            s_pair, o1, o2 = acc
            st_, sp_ = (t == 0), (t == 7)
            # col-tiled pair shares one bank and runs concurrently
            # (per-region has_written semantics verified on HW)
            nc.tensor.matmul(s_pair[0:64, :], ones_sb[:, 0:64],
                             pt[:, 0:512], start=st_, stop=sp_,
                             skip_group_check=True)
            nc.tensor.matmul(s_pair[64:128, :], ones_sb[:, 64:128],
                             pt[:, 512:1024], start=st_, stop=sp_,
                             skip_group_check=True)
            vbase = t * 1024 + j * 128
            nc.tensor.matmul(o1, vnat[:, vbase:vbase + 128], pt[:, 0:512],
                             start=st_, stop=sp_)
            nc.tensor.matmul(o2, vsw[:, vbase:vbase + 128],
                             pt[:, 512:1024], start=st_, stop=sp_)

        pending = None
        cur = None
        held = None  # t=0 consumers deferred to t=1 (slot-reuse slack)
        sts = {0: emit_st(0), 1: emit_st(1)}
        for n, (j, qi, t) in enumerate(steps):
            pt = ptp.tile([128, 1024], BF, tag="pt", name=f"pt{n}")
            nc.scalar.activation(pt, sts.pop(n), AF.Exp, scale=0.125)
            if n + 2 < len(steps):
                sts[n + 2] = emit_st(n + 2)
            if t == 0:
                if pending is not None:
                    finalize(pending)
                    pending = None
                s_pair = smp.tile([128, 512], F32, tag="sa", name=f"sp{j}{qi}")
                o1 = opp.tile([128, 512], F32, tag="ov", name=f"o1_{j}{qi}")
                o2 = opp.tile([128, 512], F32, tag="ov", name=f"o2_{j}{qi}")
                cur = (s_pair, o1, o2)
                held = (j, qi, t, pt)
                continue
            if held is not None:
                hj, hqi, ht, hpt = held
                consumers(hj, hqi, ht, hpt, cur)
                held = None
            consumers(j, qi, t, pt, cur)
            if t == 7:
                pending = (steps[n][0], steps[n][1], *cur)
        finalize(pending)

    # ---------------- O projection (partials) ----------------
    with tc.tile_pool(name="ytp", bufs=4) as ytp, \
         tc.tile_pool(name="pop", bufs=4, space="PSUM") as pop:
        for part, yt_d in ((0, ytr), (1, yti)):
            for m in range(8):
                wt = wo_tiles[(part, m)]
                for hf in range(2):
                    po = pop.tile([128, 512], F32, tag="po",
                                  name=f"po{part}{m}{hf}")
                    for jj in range(8):
                        nc.tensor.matmul(
                            po, wt[:, jj * 128:(jj + 1) * 128],
                            attn[:, jj * S + hf * 512: jj * S + (hf + 1) * 512],
                            start=(jj == 0), stop=(jj == 7))
                    yt_t = ytp.tile([128, 512], F32, tag="yt",
                                    name=f"yt{part}{m}{hf}")
                    nc.vector.tensor_copy(yt_t, po)
                    nc.sync.dma_start(
                        out=yt_d[m * 128:(m + 1) * 128,
                                 hf * 512:(hf + 1) * 512],
                        in_=yt_t)

    wop.release()
    store.release()


def build_module():
    nc = bacc.Bacc("TRN2", target_bir_lowering=False)
    with tile.TileContext(nc) as tc:
        _emit(tc)
    nc.compile()
    return nc


def _get_nc():
    if not _NC_CACHE:
        _NC_CACHE.append(build_module())
    return _NC_CACHE[0]


def prep_core(inp, core):
    """Host-side shard prep for one core (bf16 wire format)."""
    b, hg = divmod(core, 2)
    hs, he = hg * EH, (hg + 1) * EH

    def xcat(xr, xi):
        return np.ascontiguousarray(
            np.concatenate([xr[:, b, :].T, xi[:, b, :].T], axis=0)
        ).astype(BF_NP)

    def w_prep(wr, wi):
        A = wr[hs:he, :].T
        Bm = wi[hs:he, :].T
        top = np.concatenate(
            [A.reshape(E, HPC, D), Bm.reshape(E, HPC, D)], axis=2)
        bot = np.concatenate(
            [-Bm.reshape(E, HPC, D), A.reshape(E, HPC, D)], axis=2)
        return np.ascontiguousarray(
            np.concatenate([top.reshape(E, 2 * EH), bot.reshape(E, 2 * EH)],
                           axis=0)).astype(BF_NP)

    def wo_prep(w_top, w_bot):
        Ct = w_top[:, hs:he].T.reshape(HPC, D, E)
        Dt = w_bot[:, hs:he].T.reshape(HPC, D, E)
        packed = np.concatenate([Ct, Dt], axis=1).reshape(2 * EH, E)
        A2 = packed.reshape(HPC, 128, E)  # [j, p, e]
        out = np.empty((8, 128, 1024), np.float32)
        for m in range(8):
            out[m] = A2[:, :, m * 128:(m + 1) * 128].transpose(
                1, 0, 2).reshape(128, 1024)
        return out.astype(BF_NP)

    bqp = np.empty((128, HPC), np.float32)
    for j in range(HPC):
        h = hg * HPC + j
        bqp[:64, j] = inp["bq_r"][h * D:(h + 1) * D]
        bqp[64:, j] = inp["bq_i"][h * D:(h + 1) * D]

    spm = np.concatenate([np.ones((64, 1), np.float32),
                          -np.ones((64, 1), np.float32)], axis=0)

    return dict(
        xq=xcat(inp["query_r"], inp["query_i"]),
        xk=xcat(inp["key_r"], inp["key_i"]),
        xv=xcat(inp["value_r"], inp["value_i"]),
        wq=w_prep(inp["wq_r"], inp["wq_i"]),
        wk=w_prep(inp["wk_r"], inp["wk_i"]),
        wv=w_prep(inp["wv_r"], inp["wv_i"]),
        wod0=wo_prep(inp["wo_r"], -inp["wo_i"]),
        wod1=wo_prep(inp["wo_i"], inp["wo_r"]),
        bq=bqp,
        spm=spm,
        onesd=np.ones((128, 128), np.float32).astype(BF_NP),
    )


def host_combine(results, inp):
    """Sum per-core partials, add the host-side constant, untranspose."""
    bvr = inp["bv_r"].astype(np.float64)
    bvi = inp["bv_i"].astype(np.float64)
    wr = inp["wo_r"].astype(np.float64)
    wi = inp["wo_i"].astype(np.float64)
    vb_r = bvr - bvi
    vb_i = bvr + bvi
    yc_r = (wr @ vb_r - wi @ vb_i + inp["bo_r"]).astype(np.float32)
    yc_i = (wr @ vb_i + wi @ vb_r + inp["bo_i"]).astype(np.float32)

    out = np.empty((S, B, E, 2), np.float32)
    for b in range(B):
        yr = results[2 * b]["ytr"] + results[2 * b + 1]["ytr"]
        yi = results[2 * b]["yti"] + results[2 * b + 1]["yti"]
        out[:, b, :, 0] = yr.T + yc_r
        out[:, b, :, 1] = yi.T + yc_i
    return out


def kernel(**inputs):
    inputs = {k: np.asarray(v) for k, v in inputs.items()}
    nc = _get_nc()
    in_maps = [prep_core(inputs, c) for c in range(N_CORES)]
    res = run_bass_kernel_spmd(nc, in_maps, core_ids=list(range(N_CORES)))
    return host_combine(res.results, inputs)
